# revision 1
# baseline (speedup 1.0000x reference)
"""Trainium2 Bass kernel for nn_ConvBlock (sparse submanifold 3D conv + BN + ReLU).

Contract: kernel(**inputs) takes the FULL unsharded inputs (features [N,32] f32,
weight [27,32,32] f32, gamma/beta [32] f32, nbr_idx [27,N] int32, nbr_mask [27,N]
bool) and returns the FULL [N,32] f32 output.

Strategy (8 NeuronCores, SPMD):
  - Output voxels sharded across cores (50k each). Feature table replicated in
    each core's HBM as fp16 with a trailing zero row; host folds nbr_mask into
    the indices (masked -> zero row), so the device does no mask handling.
  - Per output supertile (M*128 rows) and per group of 4 kernel offsets:
    indirect-DMA gathers in the one-index-per-partition shape (offsets
    [128,1] -> dest [128,32]; the only multi-row-safe shape on TRN2 HW --
    multi-index offset APs mispair indices with descriptors), one batched
    DMA-transpose to channel-major [128(off,ch), M, 128(row)], then fp16
    matmuls accumulating all 7 groups into fp32 PSUM [32, 512] chunks.
  - BN stats (sum/sumsq per channel) partially reduced on-chip via activation
    accum_out, all-reduced across the 8 cores with a collective, then a second
    pass applies relu(a*x + b).
"""

import os
import sys

import numpy as np

sys.path.insert(0, "/opt/trn_rl_repo")

from concourse import bacc, bass, mybir, tile  # noqa: E402
from concourse.bass import IndirectOffsetOnAxis  # noqa: E402
from concourse.bass_utils import run_bass_kernel_spmd  # noqa: E402

F32 = mybir.dt.float32
F16 = mybir.dt.float16
I32 = mybir.dt.int32


class Cfg:
    def __init__(self, n_total, n_cores=8, m=32):
        self.n_total = n_total          # total voxels (BN divisor)
        self.n_cores = n_cores
        self.shard = n_total // n_cores  # valid outputs per core
        self.m = m                       # 128-row subtiles per supertile
        self.st = m * 128                # rows per supertile
        self.nst = -(-self.shard // self.st)   # supertiles per core
        self.shard_pad = self.nst * self.st
        self.zrow = n_total              # index of the zero feature row
        self.table_rows = n_total + 32   # fp16 table rows (>= zrow+1)
        self.kpad = 28                   # 27 offsets padded to 28
        self.ngroup = 7                  # 28 / 4
        self.nchunk = self.nst * max(1, self.st // min(512, self.st))  # output chunks


FULL = Cfg(400_000, 8, 32)


def build_program(cfg: Cfg):
    """Build the SPMD Bass/Tile program (same program on all cores)."""
    nc = bacc.Bacc(
        "TRN2",
        target_bir_lowering=False,
        debug=False,
        num_devices=cfg.n_cores,
    )

    table = nc.dram_tensor("table", [cfg.table_rows, 32], F16, kind="ExternalInput")
    idx_d = nc.dram_tensor(
        "idx", [cfg.nst, 128, cfg.m * cfg.kpad], I32, kind="ExternalInput"
    )
    w_d = nc.dram_tensor("w", [128, cfg.ngroup, 32], F16, kind="ExternalInput")
    gb_d = nc.dram_tensor("gb", [32, 2], F32, kind="ExternalInput")
    out_d = nc.dram_tensor("out_t", [32, cfg.shard_pad], F32, kind="ExternalOutput")
    dbg_d = nc.dram_tensor("dbg", [32, 8], F32, kind="ExternalOutput")

    conv_d = nc.dram_tensor("conv_t", [32, cfg.shard_pad], F32)
    stat_in = nc.dram_tensor("stat_in", [32, 2], F32)
    stat_out = nc.dram_tensor("stat_out", [32, 2], F32, addr_space="Shared")

    inv_n = 1.0 / float(cfg.n_total)
    eps = 1e-5
    ch = min(512, cfg.st)  # PSUM chunk free dim (<= one bank)
    bpc = ch // 128  # 128-row blocks per chunk
    nq = cfg.st // ch  # chunks per supertile
    # valid rows in the last supertile -> chunks/m-blocks actually needed
    tail_rows = cfg.shard - (cfg.nst - 1) * cfg.st
    tail_nq = -(-tail_rows // ch) if tail_rows > 0 else nq
    tail_m = tail_nq * bpc
    nchunk = (cfg.nst - 1) * nq + tail_nq  # chunks actually produced

    with tile.TileContext(nc) as tc:
        with (
            tc.tile_pool(name="constp", bufs=1) as constp,
            tc.tile_pool(name="idxp", bufs=2) as idxp,
            tc.tile_pool(name="gatherp", bufs=3) as gatherp,
            tc.tile_pool(name="trp", bufs=3) as trp,
            tc.tile_pool(name="psp", bufs=8, space="PSUM") as psp,
            tc.tile_pool(name="otp", bufs=4) as otp,
            tc.tile_pool(name="sqp", bufs=2) as sqp,
            tc.tile_pool(name="statp", bufs=1) as statp,
            tc.tile_pool(name="bnp", bufs=1) as bnp,
            tc.tile_pool(name="pbp", bufs=3) as pbp,
        ):
            # --- constants ---
            wt = constp.tile([128, cfg.ngroup * 32], F16, name="wt")
            nc.sync.dma_start(out=wt[:], in_=w_d[:, :, :])
            gbt = constp.tile([32, 2], F32, name="gbt")
            nc.sync.dma_start(out=gbt[:], in_=gb_d[:, :])

            sum_parts = statp.tile([32, nchunk], F32, name="sum_parts")
            sq_parts = statp.tile([32, nchunk], F32, name="sq_parts")

            # --- phase A: conv + partial BN stats ---
            last = cfg.nst - 1
            for s in range(cfg.nst):
                nqs = tail_nq if s == last else nq
                ms = tail_m if s == last else cfg.m
                it = idxp.tile([128, cfg.m, cfg.kpad], I32, name="it", tag="it")
                nc.sync.dma_start(out=it[:], in_=idx_d[s])

                ps_tiles = [
                    psp.tile([32, ch], F32, name=f"ps{q}", tag="ps")
                    for q in range(nqs)
                ]
                for g in range(cfg.ngroup):
                    gbuf = gatherp.tile(
                        [128, cfg.m, 4, 32], F16, name="gbuf", tag="gbuf"
                    )
                    for m in range(ms):
                        for j in range(4):
                            k = 4 * g + j
                            nc.gpsimd.indirect_dma_start(
                                out=gbuf[:, m, j, :],
                                out_offset=None,
                                in_=table[:, :],
                                in_offset=IndirectOffsetOnAxis(
                                    ap=it[:, m, k : k + 1], axis=0
                                ),
                            )
                    gt = trp.tile([128, cfg.m, 128], F16, name="gt", tag="gt")
                    nc.sync.dma_start(
                        out=gt[:, :ms, :],
                        in_=gbuf[:, :ms, :, :],
                        transpose=True,
                    )
                    for q in range(nqs):
                        nc.tensor.matmul(
                            ps_tiles[q][:],
                            wt[:, 32 * g : 32 * g + 32],
                            gt[:, bpc * q : bpc * (q + 1), :],
                            start=(g == 0),
                            stop=(g == cfg.ngroup - 1),
                        )
                for q in range(nqs):
                    c = s * nq + q
                    ot = otp.tile([32, ch], F32, name="ot", tag="ot")
                    # PSUM evacuation fused with the BN row-sum
                    nc.scalar.activation(
                        out=ot[:],
                        in_=ps_tiles[q][:],
                        func=mybir.ActivationFunctionType.Copy,
                        accum_out=sum_parts[:, c : c + 1],
                    )
                    sq = sqp.tile([32, ch], F32, name="sq", tag="sq")
                    nc.scalar.activation(
                        out=sq[:],
                        in_=ot[:],
                        func=mybir.ActivationFunctionType.Square,
                        accum_out=sq_parts[:, c : c + 1],
                    )
                    nc.sync.dma_start(
                        out=conv_d[:, s * cfg.st + ch * q : s * cfg.st + ch * (q + 1)],
                        in_=ot[:],
                    )

            # --- BN stats all-reduce ---
            stats2 = bnp.tile([32, 2], F32, name="stats2")
            nc.vector.reduce_sum(
                out=stats2[:, 0:1], in_=sum_parts[:], axis=mybir.AxisListType.X
            )
            nc.vector.reduce_sum(
                out=stats2[:, 1:2], in_=sq_parts[:], axis=mybir.AxisListType.X
            )
            nc.sync.dma_start(out=stat_in[:, :], in_=stats2[:])
            nc.gpsimd.collective_compute(
                "AllReduce",
                mybir.AluOpType.add,
                replica_groups=[list(range(cfg.n_cores))],
                ins=[stat_in[:, :]],
                outs=[stat_out[:, :]],
            )
            gstats = bnp.tile([32, 2], F32, name="gstats")
            nc.sync.dma_start(out=gstats[:], in_=stat_out[:, :])

            # a = gamma * rsqrt(var+eps); b = beta - mean * a
            mean_t = bnp.tile([32, 1], F32, name="mean_t")
            nc.vector.tensor_scalar_mul(mean_t[:], gstats[:, 0:1], inv_n)
            ex2_t = bnp.tile([32, 1], F32, name="ex2_t")
            nc.vector.tensor_scalar_mul(ex2_t[:], gstats[:, 1:2], inv_n)
            msq_t = bnp.tile([32, 1], F32, name="msq_t")
            nc.vector.tensor_tensor(
                out=msq_t[:], in0=mean_t[:], in1=mean_t[:], op=mybir.AluOpType.mult
            )
            var_t = bnp.tile([32, 1], F32, name="var_t")
            nc.vector.tensor_tensor(
                out=var_t[:], in0=ex2_t[:], in1=msq_t[:], op=mybir.AluOpType.subtract
            )
            vpe_t = bnp.tile([32, 1], F32, name="vpe_t")
            nc.vector.tensor_scalar_add(vpe_t[:], var_t[:], eps)
            std_t = bnp.tile([32, 1], F32, name="std_t")
            nc.scalar.activation(
                out=std_t[:],
                in_=vpe_t[:],
                func=mybir.ActivationFunctionType.Sqrt,
            )
            rstd_t = bnp.tile([32, 1], F32, name="rstd_t")
            nc.vector.reciprocal(rstd_t[:], std_t[:])
            a_t = bnp.tile([32, 1], F32, name="a_t")
            nc.vector.tensor_tensor(
                out=a_t[:], in0=gbt[:, 0:1], in1=rstd_t[:], op=mybir.AluOpType.mult
            )
            am_t = bnp.tile([32, 1], F32, name="am_t")
            nc.vector.tensor_tensor(
                out=am_t[:], in0=mean_t[:], in1=a_t[:], op=mybir.AluOpType.mult
            )
            b_t = bnp.tile([32, 1], F32, name="b_t")
            nc.vector.tensor_tensor(
                out=b_t[:], in0=gbt[:, 1:2], in1=am_t[:], op=mybir.AluOpType.subtract
            )

            # debug dump: stats2 (pre-AR), gstats (post-AR), mean, var, a, b
            nc.sync.dma_start(out=dbg_d[:, 0:2], in_=stats2[:])
            nc.sync.dma_start(out=dbg_d[:, 2:4], in_=gstats[:])
            nc.sync.dma_start(out=dbg_d[:, 4:5], in_=mean_t[:])
            nc.sync.dma_start(out=dbg_d[:, 5:6], in_=var_t[:])
            nc.sync.dma_start(out=dbg_d[:, 6:7], in_=a_t[:])
            nc.sync.dma_start(out=dbg_d[:, 7:8], in_=b_t[:])

            # --- phase B: y = relu(a*x + b) ---
            for s in range(cfg.nst):
                w = cfg.st if s != cfg.nst - 1 else tail_nq * ch
                xt = pbp.tile([32, cfg.st], F32, name="xt", tag="xt")
                nc.sync.dma_start(
                    out=xt[:, :w], in_=conv_d[:, s * cfg.st : s * cfg.st + w]
                )
                nc.scalar.activation(
                    out=xt[:, :w],
                    in_=xt[:, :w],
                    func=mybir.ActivationFunctionType.Relu,
                    bias=b_t[:, 0:1],
                    scale=a_t[:, 0:1],
                )
                nc.sync.dma_start(
                    out=out_d[:, s * cfg.st : s * cfg.st + w], in_=xt[:, :w]
                )

    nc.compile()
    return nc


def host_prep(cfg: Cfg, features, weight, gamma, beta, nbr_idx, nbr_mask):
    """Numpy-only input prep: fp16 table, mask folded into indices, per-core
    gather-ready index layout, stacked weights."""
    feats = np.ascontiguousarray(np.asarray(features, dtype=np.float32))
    w = np.asarray(weight, dtype=np.float32)
    idx = np.asarray(nbr_idx).astype(np.int32, copy=False)
    mask = np.asarray(nbr_mask)

    table = np.zeros((cfg.table_rows, 32), dtype=np.float16)
    table[: cfg.n_total] = feats.astype(np.float16)

    midx = np.full((cfg.kpad, cfg.shard_pad * cfg.n_cores), cfg.zrow, dtype=np.int32)
    # valid region: fold mask into indices
    folded = np.where(mask, idx, cfg.zrow).astype(np.int32)

    idx_per_core = []
    for c in range(cfg.n_cores):
        sh = np.full((cfg.kpad, cfg.shard_pad), cfg.zrow, dtype=np.int32)
        sh[:27, : cfg.shard] = folded[:, c * cfg.shard : (c + 1) * cfg.shard]
        # [kpad, nst, m, 128] -> [nst, 128, m, kpad]
        lay = sh.reshape(cfg.kpad, cfg.nst, cfg.m, 128).transpose(1, 3, 2, 0)
        idx_per_core.append(
            np.ascontiguousarray(lay).reshape(cfg.nst, 128, cfg.m * cfg.kpad)
        )

    wpad = np.zeros((cfg.kpad, 32, 32), dtype=np.float32)
    wpad[:27] = w
    # [g, j, ci, co] -> partition p = 32*j + ci, laid out [128, ngroup, 32]
    wh = np.ascontiguousarray(
        wpad.reshape(cfg.ngroup, 4, 32, 32).transpose(1, 2, 0, 3)
    ).reshape(128, cfg.ngroup, 32).astype(np.float16)

    gb = np.ascontiguousarray(
        np.stack(
            [np.asarray(gamma, np.float32), np.asarray(beta, np.float32)], axis=1
        )
    )
    return table, idx_per_core, wh, gb


_CACHE = {}
LAST_RESULTS = None


def _get_program(cfg: Cfg):
    key = (cfg.n_total, cfg.n_cores, cfg.m)
    if key not in _CACHE:
        _CACHE[key] = build_program(cfg)
    return _CACHE[key]


def kernel(features, weight, gamma, beta, nbr_idx, nbr_mask):
    global LAST_RESULTS
    cfg = FULL
    table, idx_per_core, wh, gb = host_prep(
        cfg, features, weight, gamma, beta, nbr_idx, nbr_mask
    )
    nc = _get_program(cfg)
    in_maps = [
        {"table": table, "idx": idx_per_core[c], "w": wh, "gb": gb}
        for c in range(cfg.n_cores)
    ]
    trace = bool(int(os.environ.get("CONV_TRACE", "0")))
    res = run_bass_kernel_spmd(
        nc, in_maps, list(range(cfg.n_cores)), trace=trace
    )
    LAST_RESULTS = res
    shards = [res.results[c]["out_t"][:, : cfg.shard] for c in range(cfg.n_cores)]
    out_t = np.concatenate(shards, axis=1)  # [32, N]
    return np.ascontiguousarray(out_t.T).astype(np.float32, copy=False)



# revision 6
# speedup vs baseline: 6.9465x; 6.9465x over previous
"""Trainium2 Bass kernel for nn_ConvBlock (sparse submanifold 3D conv + BN + ReLU).

Contract: kernel(**inputs) takes the FULL unsharded inputs (features [N,32] f32,
weight [27,32,32] f32, gamma/beta [32] f32, nbr_idx [27,N] int32, nbr_mask [27,N]
bool) and returns the FULL [N,32] f32 output.

Strategy (8 NeuronCores, SPMD) -- descriptor-free gather via one-hot matmuls:
  - Host reorders voxels with reverse Cuthill-McKee on the 27-neighbor graph.
    The active-voxel graph sits far below the 3D percolation threshold, so RCM
    makes every neighbor index land within ~128 rows of the diagonal: for every
    128-output tile, ALL 27x128 gathered rows come from one 256-row window.
  - Host ships, per tile: the 256-row feature window (channel-major fp16) and
    a 54-column inverse map inv[w, (k,b)] = output column using window row
    128b+w at offset k (sentinel -5 where unused; per-offset maps are
    injective so this is well-defined).
  - Device, per tile: one wide DVE is_equal against a constant iota builds all
    54 one-hot [128,128] blocks at once; a short matmul stage forms
    U = T_win @ W_k for all offsets; 54 col-tiled matmuls contract the one-hots
    with U straight into PSUM, giving the conv output with zero indirect DMA.
  - BN stats accumulated per tile (fused into the PSUM reduce + a Square
    activation), all-reduced across cores, applied in a second pass
    relu(a*x+b) exactly like the classic formulation.
"""

import os
import sys

import numpy as np

sys.path.insert(0, "/opt/trn_rl_repo")

from concourse import bacc, bass, mybir, tile  # noqa: E402
from concourse.bass_utils import run_bass_kernel_spmd  # noqa: E402

F32 = mybir.dt.float32
F16 = mybir.dt.float16

N_TOTAL = 400_000
K = 27
KP = 28            # k slots padded to 28 (4 groups of 7 for the U stage)
NB = 2             # window chunks per tile
WIN = NB * 128     # window rows per tile (256)
NO = 128           # outputs per tile
NS = K * NB        # one-hot blocks per tile (54)
N_CORES = 8
NTILE = 392        # tiles per core (392*8*128 = 401408 >= 400000)
SLAB = 8           # tiles per DMA slab
NSLAB = NTILE // SLAB
SHARD = NTILE * NO  # 50176 outputs per core


def build_program():
    nc = bacc.Bacc(
        "TRN2",
        target_bir_lowering=False,
        debug=False,
        num_devices=N_CORES,
    )

    tsl_d = nc.dram_tensor("tsl", [NSLAB, 32, SLAB * WIN], F16, kind="ExternalInput")
    inv_d = nc.dram_tensor("inv", [NSLAB, 128, SLAB * NS], F16, kind="ExternalInput")
    wg_d = nc.dram_tensor("wg", [32, KP * 32], F16, kind="ExternalInput")
    gb_d = nc.dram_tensor("gb", [32, 2], F32, kind="ExternalInput")
    out_d = nc.dram_tensor("out_t", [32, SHARD], F32, kind="ExternalOutput")
    dbg_d = nc.dram_tensor("dbg", [32, 8], F32, kind="ExternalOutput")

    conv_d = nc.dram_tensor("conv_t", [32, SHARD], F32)
    stat_in = nc.dram_tensor("stat_in", [32, 2], F32)
    stat_out = nc.dram_tensor("stat_out", [32, 2], F32, addr_space="Shared")

    inv_n = 1.0 / float(N_TOTAL)
    eps = 1e-5

    with tile.TileContext(nc) as tc:
        with (
            tc.tile_pool(name="constp", bufs=1) as constp,
            tc.tile_pool(name="inp", bufs=3) as inp,
            tc.tile_pool(name="pp", bufs=3) as pp,
            tc.tile_pool(name="up", bufs=2) as up,
            tc.tile_pool(name="ups", bufs=2, space="PSUM") as upsp,
            tc.tile_pool(name="cps", bufs=2, space="PSUM") as cps,
            tc.tile_pool(name="cvp", bufs=3) as cvp,
            tc.tile_pool(name="sqp", bufs=2) as sqp,
            tc.tile_pool(name="statp", bufs=1) as statp,
            tc.tile_pool(name="bnp", bufs=1) as bnp,
            tc.tile_pool(name="pbp", bufs=3) as pbp,
        ):
            # --- constants ---
            wg_t = constp.tile([32, KP * 32], F16, name="wg_t")
            nc.sync.dma_start(out=wg_t[:], in_=wg_d[:, :])
            gbt = constp.tile([32, 2], F32, name="gbt")
            nc.sync.dma_start(out=gbt[:], in_=gb_d[:, :])
            ocol = constp.tile([128, NS * NO], F16, name="ocol")
            nc.gpsimd.iota(
                ocol[:],
                pattern=[[0, NS], [1, NO]],
                base=0,
                channel_multiplier=0,
                allow_small_or_imprecise_dtypes=True,
            )

            sum_parts = statp.tile([32, NTILE], F32, name="sum_parts")
            sq_parts = statp.tile([32, NTILE], F32, name="sq_parts")

            # --- phase A: conv + partial BN stats ---
            per_group = [(K + 3 - j) // 4 for j in range(4)]  # k's per col group
            for s in range(NSLAB):
                tslb = inp.tile([32, SLAB * WIN], F16, name="tslb", tag="tslb")
                nc.sync.dma_start(out=tslb[:], in_=tsl_d[s])
                invb = inp.tile([128, SLAB * NS], F16, name="invb", tag="invb")
                nc.sync.dma_start(out=invb[:], in_=inv_d[s])
                cvb = cvp.tile([32, SLAB * NO], F32, name="cvb", tag="cvb")

                for q in range(SLAB):
                    t = s * SLAB + q
                    # one-hot: P[w, (k,b,o)] = (o == inv[w, (k,b)])
                    P = pp.tile([128, NS * NO], F16, name="P", tag="P")
                    nc.vector.tensor_tensor(
                        out=P[:].rearrange("p (s o) -> p s o", s=NS),
                        in0=ocol[:].rearrange("p (s o) -> p s o", s=NS),
                        in1=invb[:, q * NS : (q + 1) * NS]
                        .unsqueeze(2)
                        .broadcast_to([128, NS, NO]),
                        op=mybir.AluOpType.is_equal,
                    )
                    # U stage: U[w, (b,k,co)] = (T_chunk_b^T @ W)[w, (k,co)]
                    u_sb = up.tile([128, NB * KP * 32], F16, name="u_sb", tag="u_sb")
                    for b in range(NB):
                        lhsT = tslb[:, q * WIN + 128 * b : q * WIN + 128 * (b + 1)]
                        ua = upsp.tile([128, 512], F32, name="ua", tag="ua")
                        nc.tensor.matmul(
                            ua[:], lhsT, wg_t[:, 0:512], start=True, stop=True
                        )
                        ub = upsp.tile([128, 384], F32, name="ub", tag="ub")
                        nc.tensor.matmul(
                            ub[:], lhsT, wg_t[:, 512:896], start=True, stop=True
                        )
                        off = b * KP * 32
                        nc.scalar.activation(
                            out=u_sb[:, off : off + 512],
                            in_=ua[:],
                            func=mybir.ActivationFunctionType.Copy,
                        )
                        nc.scalar.activation(
                            out=u_sb[:, off + 512 : off + 896],
                            in_=ub[:],
                            func=mybir.ActivationFunctionType.Copy,
                        )
                    # main: col-tiled one-hot contraction into conv PSUM
                    conv4 = cps.tile([128, NO], F32, name="conv4", tag="conv4")
                    seen = [0] * 4
                    for k in range(K):
                        j = k % 4
                        for b in range(NB):
                            blk = k * NB + b
                            seen[j] += 1
                            nc.tensor.matmul(
                                conv4[32 * j : 32 * (j + 1), :],
                                u_sb[:, b * KP * 32 + 32 * k : b * KP * 32 + 32 * (k + 1)],
                                P[:, blk * NO : (blk + 1) * NO],
                                start=(seen[j] == 1),
                                stop=(seen[j] == per_group[j] * NB),
                                tile_position=(0, 32 * j),
                                skip_group_check=True,
                            )
                    # reduce col groups + fused BN sum; square for sumsq
                    t0 = sqp.tile([32, NO], F32, name="t0", tag="t0")
                    nc.vector.tensor_copy(t0[:], conv4[0:32, :])
                    nc.vector.tensor_tensor(
                        out=t0[:], in0=t0[:], in1=conv4[32:64, :],
                        op=mybir.AluOpType.add,
                    )
                    nc.vector.tensor_tensor(
                        out=t0[:], in0=t0[:], in1=conv4[64:96, :],
                        op=mybir.AluOpType.add,
                    )
                    cv = cvb[:, q * NO : (q + 1) * NO]
                    nc.vector.scalar_tensor_tensor(
                        out=cv, in0=t0[:], scalar=1.0, in1=conv4[96:128, :],
                        op0=mybir.AluOpType.mult, op1=mybir.AluOpType.add,
                        accum_out=sum_parts[:, t : t + 1],
                    )
                    sq = sqp.tile([32, NO], F32, name="sq", tag="sq")
                    nc.scalar.activation(
                        out=sq[:], in_=cv,
                        func=mybir.ActivationFunctionType.Square,
                        accum_out=sq_parts[:, t : t + 1],
                    )
                nc.sync.dma_start(
                    out=conv_d[:, s * SLAB * NO : (s + 1) * SLAB * NO], in_=cvb[:]
                )

            # --- BN stats all-reduce ---
            stats2 = bnp.tile([32, 2], F32, name="stats2")
            nc.vector.reduce_sum(
                out=stats2[:, 0:1], in_=sum_parts[:], axis=mybir.AxisListType.X
            )
            nc.vector.reduce_sum(
                out=stats2[:, 1:2], in_=sq_parts[:], axis=mybir.AxisListType.X
            )
            nc.sync.dma_start(out=stat_in[:, :], in_=stats2[:])
            nc.gpsimd.collective_compute(
                "AllReduce",
                mybir.AluOpType.add,
                replica_groups=[list(range(N_CORES))],
                ins=[stat_in[:, :]],
                outs=[stat_out[:, :]],
            )
            gstats = bnp.tile([32, 2], F32, name="gstats")
            nc.sync.dma_start(out=gstats[:], in_=stat_out[:, :])

            # a = gamma * rsqrt(var+eps); b = beta - mean * a
            mean_t = bnp.tile([32, 1], F32, name="mean_t")
            nc.vector.tensor_scalar_mul(mean_t[:], gstats[:, 0:1], inv_n)
            ex2_t = bnp.tile([32, 1], F32, name="ex2_t")
            nc.vector.tensor_scalar_mul(ex2_t[:], gstats[:, 1:2], inv_n)
            msq_t = bnp.tile([32, 1], F32, name="msq_t")
            nc.vector.tensor_tensor(
                out=msq_t[:], in0=mean_t[:], in1=mean_t[:], op=mybir.AluOpType.mult
            )
            var_t = bnp.tile([32, 1], F32, name="var_t")
            nc.vector.tensor_tensor(
                out=var_t[:], in0=ex2_t[:], in1=msq_t[:], op=mybir.AluOpType.subtract
            )
            vpe_t = bnp.tile([32, 1], F32, name="vpe_t")
            nc.vector.tensor_scalar_add(vpe_t[:], var_t[:], eps)
            std_t = bnp.tile([32, 1], F32, name="std_t")
            nc.scalar.activation(
                out=std_t[:], in_=vpe_t[:],
                func=mybir.ActivationFunctionType.Sqrt,
            )
            rstd_t = bnp.tile([32, 1], F32, name="rstd_t")
            nc.vector.reciprocal(rstd_t[:], std_t[:])
            a_t = bnp.tile([32, 1], F32, name="a_t")
            nc.vector.tensor_tensor(
                out=a_t[:], in0=gbt[:, 0:1], in1=rstd_t[:], op=mybir.AluOpType.mult
            )
            am_t = bnp.tile([32, 1], F32, name="am_t")
            nc.vector.tensor_tensor(
                out=am_t[:], in0=mean_t[:], in1=a_t[:], op=mybir.AluOpType.mult
            )
            b_t = bnp.tile([32, 1], F32, name="b_t")
            nc.vector.tensor_tensor(
                out=b_t[:], in0=gbt[:, 1:2], in1=am_t[:], op=mybir.AluOpType.subtract
            )

            nc.sync.dma_start(out=dbg_d[:, 0:2], in_=stats2[:])
            nc.sync.dma_start(out=dbg_d[:, 2:4], in_=gstats[:])
            nc.sync.dma_start(out=dbg_d[:, 4:5], in_=mean_t[:])
            nc.sync.dma_start(out=dbg_d[:, 5:6], in_=var_t[:])
            nc.sync.dma_start(out=dbg_d[:, 6:7], in_=a_t[:])
            nc.sync.dma_start(out=dbg_d[:, 7:8], in_=b_t[:])

            # --- phase B: y = relu(a*x + b) ---
            CB = 4096
            nb_steps = -(-SHARD // CB)
            for i in range(nb_steps):
                w = min(CB, SHARD - i * CB)
                xt = pbp.tile([32, CB], F32, name="xt", tag="xt")
                nc.sync.dma_start(
                    out=xt[:, :w], in_=conv_d[:, i * CB : i * CB + w]
                )
                nc.scalar.activation(
                    out=xt[:, :w],
                    in_=xt[:, :w],
                    func=mybir.ActivationFunctionType.Relu,
                    bias=b_t[:, 0:1],
                    scale=a_t[:, 0:1],
                )
                nc.sync.dma_start(
                    out=out_d[:, i * CB : i * CB + w], in_=xt[:, :w]
                )

    nc.compile()
    return nc


def host_prep(features, weight, gamma, beta, nbr_idx, nbr_mask):
    """Numpy-only prep: RCM ordering, per-tile windows + inverse maps."""
    from scipy import sparse
    from scipy.sparse.csgraph import reverse_cuthill_mckee

    feats = np.asarray(features, dtype=np.float32)
    w = np.asarray(weight, dtype=np.float32)
    idx = np.asarray(nbr_idx).astype(np.int64, copy=False)
    mask = np.asarray(nbr_mask)
    N = feats.shape[0]
    assert N == N_TOTAL

    rows = np.repeat(np.arange(N)[None, :], K, axis=0)[mask]
    cols = idx[mask]
    A = sparse.csr_matrix(
        (np.ones(len(rows), np.int8), (rows, cols)), shape=(N, N)
    )
    perm = np.asarray(reverse_cuthill_mckee(A, symmetric_mode=True), dtype=np.int64)
    iperm = np.empty(N, np.int64)
    iperm[perm] = np.arange(N)

    idx_s = np.where(mask[:, perm], iperm[idx[:, perm]], -1)  # [K, N]

    ntile_g = NTILE * N_CORES  # 3136
    npad = ntile_g * NO - N
    idx_sp = np.concatenate(
        [idx_s, np.full((K, npad), -1, dtype=np.int64)], axis=1
    ).reshape(K, ntile_g, NO)

    # per-tile window base
    valid = idx_sp >= 0
    big = np.where(valid, idx_sp, np.iinfo(np.int64).max)
    sml = np.where(valid, idx_sp, -1)
    lo = big.min(axis=(0, 2))
    hi = sml.max(axis=(0, 2))
    has = valid.any(axis=(0, 2))
    w0 = np.where(has, lo, 0)
    width = np.where(has, hi - lo + 1, 1)
    assert width.max() <= WIN, f"tile window too wide: {width.max()}"

    # window slices, channel-major fp16
    table = feats[perm].astype(np.float16)  # [N, 32] sorted
    table_pad = np.concatenate(
        [table, np.zeros((WIN, 32), np.float16)], axis=0
    )
    gidx = w0[:, None] + np.arange(WIN)[None, :]  # [ntile_g, WIN]
    gidx = np.minimum(gidx, N + WIN - 1)
    tsl_all = np.ascontiguousarray(
        table_pad[gidx].transpose(0, 2, 1)
    )  # [ntile_g, 32, WIN]

    # inverse maps
    inv_all = np.full((ntile_g, 128, NS), -5.0, dtype=np.float16)
    kk, tt, oo = np.nonzero(valid)
    ww = idx_sp[kk, tt, oo] - w0[tt]
    assert ww.min() >= 0 and ww.max() < WIN
    bb = ww >> 7
    wi = ww & 127
    inv_all[tt, wi, kk * NB + bb] = oo.astype(np.float16)

    # per-core slab layouts
    tsl_cores, inv_cores = [], []
    for c in range(N_CORES):
        ts = tsl_all[c * NTILE : (c + 1) * NTILE]  # [NTILE, 32, WIN]
        ts = np.ascontiguousarray(
            ts.reshape(NSLAB, SLAB, 32, WIN).transpose(0, 2, 1, 3)
        ).reshape(NSLAB, 32, SLAB * WIN)
        tsl_cores.append(ts)
        iv = inv_all[c * NTILE : (c + 1) * NTILE]  # [NTILE, 128, NS]
        iv = np.ascontiguousarray(
            iv.reshape(NSLAB, SLAB, 128, NS).transpose(0, 2, 1, 3)
        ).reshape(NSLAB, 128, SLAB * NS)
        inv_cores.append(iv)

    wg = np.zeros((32, KP * 32), dtype=np.float16)
    wg[:, : K * 32] = w.transpose(1, 0, 2).reshape(32, K * 32).astype(np.float16)

    gb = np.ascontiguousarray(
        np.stack(
            [np.asarray(gamma, np.float32), np.asarray(beta, np.float32)], axis=1
        )
    )
    return tsl_cores, inv_cores, wg, gb, perm


_CACHE = {}
LAST_RESULTS = None


def _get_program():
    if "nc" not in _CACHE:
        _CACHE["nc"] = build_program()
    return _CACHE["nc"]


def kernel(features, weight, gamma, beta, nbr_idx, nbr_mask):
    global LAST_RESULTS
    tsl_cores, inv_cores, wg, gb, perm = host_prep(
        features, weight, gamma, beta, nbr_idx, nbr_mask
    )
    nc = _get_program()
    in_maps = [
        {"tsl": tsl_cores[c], "inv": inv_cores[c], "wg": wg, "gb": gb}
        for c in range(N_CORES)
    ]
    trace = bool(int(os.environ.get("CONV_TRACE", "0")))
    res = run_bass_kernel_spmd(nc, in_maps, list(range(N_CORES)), trace=trace)
    LAST_RESULTS = res
    shards = [res.results[c]["out_t"] for c in range(N_CORES)]
    sorted_out = np.concatenate(shards, axis=1)[:, :N_TOTAL]  # [32, N]
    out = np.empty((N_TOTAL, 32), dtype=np.float32)
    out[perm] = sorted_out.T
    return out
